# revision 20
# baseline (speedup 1.0000x reference)
"""ColorUnpool (gather + segment-max + relu) as an 8-core Trainium2 Bass kernel.

Reference semantics:
    out = zeros([200000, 256]);  out[center_idx] = feat            # centers
    seg = segment_max(feat[edge_src], edge_dst)                    # edges
    out[r] = max(seg[r], 0) for rows r with >= 1 incoming edge

edge_dst only hits rows [50000, 200000) and center_idx only [0, 50000), so
the two regions are disjoint.  The center region is a pure host-side copy of
the input (no compute); the device computes the edge region only.

Device strategy (per core, dst rows split 8 ways -> 18750 rows/core):
  * Rows are degree-sorted (desc) and packed into 147 tiles of 128 rows.
    Column layout is round-major: round 0 holds one column per tile (edge 0
    of every row, ZID pad for deg-0 rows); round j>=1 holds a column for
    each tile whose max degree exceeds j (a prefix, since tiles are sorted).
  * The feat table is compacted per core to its ~31.6k distinct src rows
    (< 32768), so gather indices fit in int16 and the gather runs as
    1024-index `dma_gather` instructions (the HW cap) round-robined over
    all 4 SWDGE queues -- descriptor generation for different queues runs
    concurrently on the Q7 cores, which quarters the ~7.7ns/row software
    DGE cost that serialized the old per-column indirect-DMA design.
  * Round 0 gathers straight into the accumulator; rounds j>=1 gather into
    rotating SBUF chunks and fold in with fused DVE ops
    acc = max(max(acc, 0), g)  (scalar_tensor_tensor), which also bakes in
    the final relu.  Tiles only touched by round 0 get an Activation-engine
    relu instead.  Finished tile ranges are written back to DRAM as soon as
    their last round completes, overlapping the output DMA with the
    remaining gathers.
  * feat is bf16 on device (rel err ~4e-3 << 2e-2 gate); the host
    un-permutes rows and upcasts to f32.
"""

import sys
import types

import numpy as np
import ml_dtypes

sys.path.insert(0, "/opt/trn_rl_repo")

N_NODES = 200000
N_CENTERS = 50000
FEAT = 256
NCORES = 8
P = 128

R_EDGE = N_NODES - N_CENTERS          # 150000 edge-target rows
RC = R_EDGE // NCORES                 # 18750 edge rows per core
TILES = (RC + P - 1) // P             # 147 tiles of 128 rows
NPOS = TILES * P                      # 18816 padded row slots
TBL = 32768                           # per-core compact feat table rows
ZID = TBL - 1                         # zero row id (table is zero-padded)
G = 8                                 # gather chunk width (cols); HW caps a
                                      # single dma_gather at 1024 indices
WMIN = 8                              # min writeback width (tiles)


def _install_profile_hook():
    """Provide antenv.axon_hooks (missing on this image) so that
    run_bass_kernel_spmd(trace=True) can profile via the axon .so."""
    try:
        import antenv
        if "antenv.axon_hooks" in sys.modules:
            return
        from trn_agent_boot.trn_boot import _ntff_profile_via_ctypes
        mod = types.ModuleType("antenv.axon_hooks")
        hook = _ntff_profile_via_ctypes("/opt/axon/libaxon_pjrt.so")
        mod.get_axon_ntff_profile_hook = lambda: hook
        mod.set_axon_ntff_profile_hook = lambda h: None
        sys.modules["antenv.axon_hooks"] = mod
        antenv.axon_hooks = mod
    except Exception:
        pass


def _build_plan(edge_src, edge_dst, feat):
    """Host preprocessing.

    Returns (T, bases, C, tables, idx_planes, orders):
      T          = per-round union active-tile counts, T[0] == TILES
      bases      = column base per round
      C          = total columns
      tables     = per-core compact bf16 feat tables [TBL, FEAT]
      idx_planes = per-core int16 idx planes [P, C*8] (x8 Q7 replication)
      orders     = per-core position->local-row permutation [RC]
    """
    edge_src = np.asarray(edge_src, np.int64)
    edge_dst = np.asarray(edge_dst, np.int64)
    local_dst = edge_dst - N_CENTERS
    assert local_dst.min() >= 0 and local_dst.max() < R_EDGE
    core_of = local_dst // RC

    percore = []
    for c in range(NCORES):
        m = core_of == c
        ld = (local_dst[m] % RC).astype(np.int64)
        ss = edge_src[m].astype(np.int64)
        deg = np.bincount(ld, minlength=RC)
        order = np.argsort(-deg, kind="stable")          # rows desc by degree
        eo = np.argsort(ld, kind="stable")
        ss_sorted = ss[eo]                               # CSR values
        starts = np.concatenate([[0], np.cumsum(deg)[:-1]])
        uniq, inv = np.unique(ss_sorted, return_inverse=True)
        assert len(uniq) < TBL, f"core {c}: {len(uniq)} distinct srcs > int16"
        ssc = inv.astype(np.int64)                       # compact CSR values
        deg_sorted = deg[order]
        d_tile = deg_sorted[np.arange(TILES) * P]        # per-tile max degree
        percore.append(dict(deg=deg, order=order, ssc=ssc, starts=starts,
                            d_tile=d_tile, uniq=uniq))

    maxd = max(max(int(pc["d_tile"][0]), 1) for pc in percore)
    T = [TILES]                                          # round 0: all tiles
    for j in range(1, maxd):
        T.append(max(int((pc["d_tile"] > j).sum()) for pc in percore))
    T1 = T[1] if maxd > 1 else 0
    # segment order: round 0 of deg>=2 tiles, all reduction rounds, then
    # round 0 of deg<=1 tiles LAST -- the final chunks need no DVE chain
    # (just Act relu + write), which collapses the post-gather tail
    segs = [(0, 0, T1)]
    segs += [(j, 0, T[j]) for j in range(1, maxd)]
    segs += [(0, T1, TILES - T1)]
    segs = [s for s in segs if s[2] > 0]
    col_base = np.concatenate([[0], np.cumsum([n for _, _, n in segs])])
    C = int(col_base[-1])

    tables, idx_planes, orders = [], [], []
    for pc in percore:
        order_padded = np.full(NPOS, -1, np.int64)
        order_padded[:RC] = pc["order"]
        deg, starts, ssc = pc["deg"], pc["starts"], pc["ssc"]
        vals = np.full(C * P, ZID, np.int64)
        for si, (j, t0, n) in enumerate(segs):
            qpos = np.arange(t0 * P, (t0 + n) * P)
            r = order_padded[qpos]
            rs = np.where(r >= 0, r, 0)
            has = (r >= 0) & (deg[rs] > j)
            v = np.where(has, ssc[np.minimum(starts[rs] + j, len(ssc) - 1)],
                         ZID)
            base = int(col_base[si]) * P
            vals[base:base + n * P] = v
        # idx position g lives at [g%16, g//16], replicated x8 for Q7 cores
        plane16 = vals.astype(np.int16).reshape(C * 8, 16).T
        idx_planes.append(np.ascontiguousarray(np.tile(plane16, (8, 1))))
        tbl = np.zeros((TBL, FEAT), ml_dtypes.bfloat16)
        tbl[:len(pc["uniq"])] = feat[pc["uniq"]].astype(ml_dtypes.bfloat16)
        tables.append(tbl)
        orders.append(pc["order"])
    return T, segs, col_base, C, tables, idx_planes, orders


def _build_bass(T, segs, col_base, C):
    import concourse.bacc as bacc
    import concourse.mybir as mybir
    import concourse.tile as tile

    maxd = len(T)
    nc = bacc.Bacc("TRN2", target_bir_lowering=False, debug=False,
                   num_devices=NCORES, num_swdge_queues=4)
    t_feat = nc.dram_tensor("feat_tbl", [TBL, FEAT], mybir.dt.bfloat16,
                            kind="ExternalInput")
    t_idx = nc.dram_tensor("idxs", [P, C * 8], mybir.dt.int16,
                           kind="ExternalInput")
    t_oe = nc.dram_tensor("out_edge", [P, TILES, FEAT], mybir.dt.bfloat16,
                          kind="ExternalOutput")

    mx = mybir.AluOpType.max
    relu = mybir.ActivationFunctionType.Relu

    # G-column chunks, split at acc-direct (round 0) segment boundaries
    bounds = [0, C]
    for si, (j, _, _) in enumerate(segs):
        if j == 0:
            bounds += [int(col_base[si]), int(col_base[si + 1])]
    chunks = []
    for lo, hi in zip(sorted(set(bounds))[:-1], sorted(set(bounds))[1:]):
        for s in range(lo, hi, G):
            chunks.append((s, min(s + G, hi)))



    with tile.TileContext(nc) as tc:
        with tc.tile_pool(name="idxp", bufs=1) as idxp, \
             tc.tile_pool(name="accp", bufs=1) as accp, \
             tc.tile_pool(name="gp", bufs=8) as gp:
            idx = idxp.tile([P, C * 8], mybir.dt.int16)
            # dummy 16-idx gather with no data deps: triggers the Q7 mlp
            # library IRAM load (~8us) during the preamble/idx load instead
            # of stalling the first real gather
            idxw = idxp.tile([P, 1], mybir.dt.int16)
            nc.gpsimd.memset(idxw[:], 0)
            warm = idxp.tile([P, 1, FEAT], mybir.dt.bfloat16)
            nc.gpsimd.dma_gather(warm[:], t_feat[:], idxw[:], 16, 16, FEAT,
                                 queue_num=0)
            nc.sync.dma_start(out=idx[:], in_=t_idx[:])
            acc = accp.tile([P, TILES, FEAT], mybir.dt.bfloat16)

            pend = []          # pending finalized tile ranges [lo, hi)

            def add_final(lo, hi, force=False):
                if lo < hi:
                    if pend and pend[-1][1] == lo:
                        pend[-1] = (pend[-1][0], hi)
                    elif pend and pend[-1][0] == hi:
                        pend[-1] = (lo, pend[-1][1])
                    else:
                        pend.append((lo, hi))
                keep = []
                for lo, hi in pend:
                    if hi - lo >= WMIN or force:
                        nc.sync.dma_start(out=t_oe[:, lo:hi, :],
                                          in_=acc[:, lo:hi, :])
                    else:
                        keep.append((lo, hi))
                pend[:] = keep

            # chunk -> list of (segment, piece) handling
            seg_rng = [(int(col_base[si]), int(col_base[si + 1]))
                       for si in range(len(segs))]
            for k, (cs, ce) in enumerate(chunks):
                w = ce - cs
                pieces = []
                direct = None
                for si, (j, t0s, n) in enumerate(segs):
                    a = max(cs, seg_rng[si][0])
                    b = min(ce, seg_rng[si][1])
                    if a < b:
                        pieces.append((si, j, t0s, a, b))
                        if j == 0:
                            direct = (t0s + (a - seg_rng[si][0]),
                                      t0s + (b - seg_rng[si][0]))
                if direct is not None:                   # round 0: direct
                    assert len(pieces) == 1
                    gout = acc[:, direct[0]:direct[1], :]
                else:
                    g = gp.tile([P, G, FEAT], mybir.dt.bfloat16, tag="g")
                    gout = g[:, :w, :]
                nc.gpsimd.dma_gather(gout, t_feat[:], idx[:, cs * 8:ce * 8],
                                     w * P, w * P, FEAT,
                                     queue_num=(k + 1) % 4)
                for si, j, t0s, a, b in pieces:
                    nxt = T[j + 1] if j + 1 < maxd else 0
                    if j == 0:
                        lo, hi = direct
                        if t0s > 0:
                            # deg<=1 tiles: relu on Act engine, then final
                            nc.scalar.activation(acc[:, lo:hi, :],
                                                 acc[:, lo:hi, :], relu)
                            add_final(lo, hi)
                        continue
                    tp = a - seg_rng[si][0]
                    L = b - a
                    if j == 1:
                        # round 1 touches every deg>=2 tile exactly once:
                        # fold the relu in; later rounds use plain max
                        nc.vector.scalar_tensor_tensor(
                            out=acc[:, tp:tp + L, :],
                            in0=acc[:, tp:tp + L, :], scalar=0.0,
                            in1=g[:, a - cs:b - cs, :], op0=mx, op1=mx)
                    else:
                        nc.vector.tensor_tensor(
                            out=acc[:, tp:tp + L, :],
                            in0=acc[:, tp:tp + L, :],
                            in1=g[:, a - cs:b - cs, :], op=mx)
                    add_final(max(tp, nxt), tp + L)
            add_final(0, 0, force=True)
    nc.compile()
    return nc


def _unshard(results, orders, feat_centers):
    out = np.empty((N_NODES, FEAT), np.float32)
    out[:N_CENTERS] = feat_centers                       # centers: exact copy
    for c in range(NCORES):
        oe = np.asarray(results[c]["out_edge"])          # [P, TILES, FEAT]
        vals = oe.transpose(1, 0, 2).reshape(NPOS, FEAT)  # position-major
        rows = N_CENTERS + c * RC + orders[c]            # position q -> row
        out[rows] = vals[:RC].astype(np.float32)
    return out


def kernel(feat, center_idx, edge_src, edge_dst, n_nodes, _trace=False):
    assert int(n_nodes) == N_NODES
    feat = np.ascontiguousarray(np.asarray(feat, np.float32))
    center_idx = np.asarray(center_idx, np.int64)

    # centers: out[center_idx] = feat, handled fully on the host (pure copy)
    feat_centers = np.zeros((N_CENTERS, FEAT), np.float32)
    feat_centers[center_idx] = feat

    T, segs, col_base, C, tables, idx_planes, orders = _build_plan(
        edge_src, edge_dst, feat)
    nc = _build_bass(T, segs, col_base, C)

    if _trace:
        _install_profile_hook()
    import concourse.bass_utils as bass_utils
    bass_utils.upload_artifacts = lambda tmpdir: f"file://{tmpdir}"
    from concourse.bass_utils import run_bass_kernel_spmd

    in_maps = [{"feat_tbl": tables[c], "idxs": idx_planes[c]}
               for c in range(NCORES)]
    kw = dict(trace=True) if _trace else {}
    res = run_bass_kernel_spmd(nc, in_maps, list(range(NCORES)), **kw)

    out = _unshard(res.results, orders, feat_centers)
    if _trace:
        return out, res
    return out


# revision 25
# speedup vs baseline: 1.0533x; 1.0533x over previous
"""ColorUnpool (gather + segment-max + relu) as an 8-core Trainium2 Bass kernel.

Reference semantics:
    out = zeros([200000, 256]);  out[center_idx] = feat            # centers
    seg = segment_max(feat[edge_src], edge_dst)                    # edges
    out[r] = max(seg[r], 0) for rows r with >= 1 incoming edge

edge_dst only hits rows [50000, 200000) and center_idx only [0, 50000), so
the two regions are disjoint.  The center region is a pure host-side copy of
the input (no compute); the device computes the edge region only.

Device strategy (per core, dst rows split 8 ways -> 18750 rows/core):
  * Rows are degree-sorted (desc) and packed into 147 tiles of 128 rows.
    Column layout is round-major: round 0 holds one column per tile (edge 0
    of every row, ZID pad for deg-0 rows); round j>=1 holds a column for
    each tile whose max degree exceeds j (a prefix, since tiles are sorted).
  * The feat table is compacted per core to its ~31.6k distinct src rows
    (< 32768), so gather indices fit in int16 and the gather runs as
    1024-index `dma_gather` instructions (the HW cap) round-robined over
    all 4 SWDGE queues -- descriptor generation for different queues runs
    concurrently on the Q7 cores, which quarters the ~7.7ns/row software
    DGE cost that serialized the old per-column indirect-DMA design
    (the baseline's bottleneck: 412 x 994ns of Pool-engine SWDGE time).
  * A dummy 16-idx gather with no data deps runs first so the Q7 mlp
    library IRAM load overlaps the preamble and the idx-plane load.
  * Round 0 gathers straight into the accumulator; rounds j>=1 gather into
    rotating SBUF chunks and fold in with DVE max ops.  Round 1 (which
    touches every deg>=2 tile exactly once) uses the fused
    scalar_tensor_tensor  acc = max(max(acc, 0), g), baking in the final
    relu; later rounds use the faster plain tensor_tensor max.  Tiles only
    touched by round 0 get an Activation-engine relu instead.  Finished
    tile ranges are written back to DRAM as soon as their last round
    completes, overlapping the output DMA with the remaining gathers.
  * feat is bf16 on device (rel err ~4e-3 << 2e-2 gate); the host
    un-permutes rows and upcasts to f32.
"""

import sys
import types

import numpy as np
import ml_dtypes

sys.path.insert(0, "/opt/trn_rl_repo")

N_NODES = 200000
N_CENTERS = 50000
FEAT = 256
NCORES = 8
P = 128

R_EDGE = N_NODES - N_CENTERS          # 150000 edge-target rows
RC = R_EDGE // NCORES                 # 18750 edge rows per core
TILES = (RC + P - 1) // P             # 147 tiles of 128 rows
NPOS = TILES * P                      # 18816 padded row slots
TBL = 32768                           # per-core compact feat table rows
ZID = TBL - 1                         # zero row id (table is zero-padded)
G = 8                                 # gather chunk width (cols); HW caps a
                                      # single dma_gather at 1024 indices
WMIN = 8                              # min writeback width (tiles)


def _install_profile_hook():
    """Provide antenv.axon_hooks (missing on this image) so that
    run_bass_kernel_spmd(trace=True) can profile via the axon .so."""
    try:
        import antenv
        if "antenv.axon_hooks" in sys.modules:
            return
        from trn_agent_boot.trn_boot import _ntff_profile_via_ctypes
        mod = types.ModuleType("antenv.axon_hooks")
        hook = _ntff_profile_via_ctypes("/opt/axon/libaxon_pjrt.so")
        mod.get_axon_ntff_profile_hook = lambda: hook
        mod.set_axon_ntff_profile_hook = lambda h: None
        sys.modules["antenv.axon_hooks"] = mod
        antenv.axon_hooks = mod
    except Exception:
        pass


def _build_plan(edge_src, edge_dst, feat):
    """Host preprocessing.

    Returns (T, bases, C, tables, idx_planes, orders):
      T          = per-round union active-tile counts, T[0] == TILES
      bases      = column base per round
      C          = total columns
      tables     = per-core compact bf16 feat tables [TBL, FEAT]
      idx_planes = per-core int16 idx planes [P, C*8] (x8 Q7 replication)
      orders     = per-core position->local-row permutation [RC]
    """
    edge_src = np.asarray(edge_src, np.int64)
    edge_dst = np.asarray(edge_dst, np.int64)
    local_dst = edge_dst - N_CENTERS
    assert local_dst.min() >= 0 and local_dst.max() < R_EDGE
    core_of = local_dst // RC

    percore = []
    for c in range(NCORES):
        m = core_of == c
        ld = (local_dst[m] % RC).astype(np.int64)
        ss = edge_src[m].astype(np.int64)
        deg = np.bincount(ld, minlength=RC)
        order = np.argsort(-deg, kind="stable")          # rows desc by degree
        eo = np.argsort(ld, kind="stable")
        ss_sorted = ss[eo]                               # CSR values
        starts = np.concatenate([[0], np.cumsum(deg)[:-1]])
        uniq, inv = np.unique(ss_sorted, return_inverse=True)
        assert len(uniq) < TBL, f"core {c}: {len(uniq)} distinct srcs > int16"
        ssc = inv.astype(np.int64)                       # compact CSR values
        deg_sorted = deg[order]
        d_tile = deg_sorted[np.arange(TILES) * P]        # per-tile max degree
        percore.append(dict(deg=deg, order=order, ssc=ssc, starts=starts,
                            d_tile=d_tile, uniq=uniq))

    maxd = max(max(int(pc["d_tile"][0]), 1) for pc in percore)
    T = [TILES]                                          # round 0: all tiles
    for j in range(1, maxd):
        T.append(max(int((pc["d_tile"] > j).sum()) for pc in percore))
    bases = np.concatenate([[0], np.cumsum(T)[:-1]]).astype(int)
    C = int(np.sum(T))

    tables, idx_planes, orders = [], [], []
    for pc in percore:
        order_padded = np.full(NPOS, -1, np.int64)
        order_padded[:RC] = pc["order"]
        deg, starts, ssc = pc["deg"], pc["starts"], pc["ssc"]
        vals = np.full(C * P, ZID, np.int64)
        for j in range(maxd):
            qpos = np.arange(T[j] * P)
            r = order_padded[qpos]
            rs = np.where(r >= 0, r, 0)
            has = (r >= 0) & (deg[rs] > j)
            v = np.where(has, ssc[np.minimum(starts[rs] + j, len(ssc) - 1)],
                         ZID)
            vals[bases[j] * P: bases[j] * P + T[j] * P] = v
        # idx position g lives at [g%16, g//16], replicated x8 for Q7 cores
        plane16 = vals.astype(np.int16).reshape(C * 8, 16).T
        idx_planes.append(np.ascontiguousarray(np.tile(plane16, (8, 1))))
        tbl = np.zeros((TBL, FEAT), ml_dtypes.bfloat16)
        tbl[:len(pc["uniq"])] = feat[pc["uniq"]].astype(ml_dtypes.bfloat16)
        tables.append(tbl)
        orders.append(pc["order"])
    return T, bases, C, tables, idx_planes, orders


def _build_bass(T, bases, C):
    import concourse.bacc as bacc
    import concourse.mybir as mybir
    import concourse.tile as tile

    maxd = len(T)
    nc = bacc.Bacc("TRN2", target_bir_lowering=False, debug=False,
                   num_devices=NCORES, num_swdge_queues=4)
    t_feat = nc.dram_tensor("feat_tbl", [TBL, FEAT], mybir.dt.bfloat16,
                            kind="ExternalInput")
    t_idx = nc.dram_tensor("idxs", [P, C * 8], mybir.dt.int16,
                           kind="ExternalInput")
    t_oe = nc.dram_tensor("out_edge", [P, TILES, FEAT], mybir.dt.bfloat16,
                          kind="ExternalOutput")

    mx = mybir.AluOpType.max
    relu = mybir.ActivationFunctionType.Relu

    # G-column chunks, split at the round-0 boundary (those go straight
    # into the accumulator)
    chunks = []
    s = 0
    while s < C:
        e = min(s + G, TILES if s < TILES else C)
        chunks.append((s, e))
        s = e

    # last chunk index per round (where its final column lands)
    end_chunk = {}
    for j in range(maxd):
        last_col = bases[j] + T[j] - 1
        for k, (cs, ce) in enumerate(chunks):
            if cs <= last_col < ce:
                end_chunk[j] = k

    with tile.TileContext(nc) as tc:
        with tc.tile_pool(name="idxp", bufs=1) as idxp, \
             tc.tile_pool(name="accp", bufs=1) as accp, \
             tc.tile_pool(name="gp", bufs=8) as gp:
            idx = idxp.tile([P, C * 8], mybir.dt.int16)
            # dummy 16-idx gather with no data deps: triggers the Q7 mlp
            # library IRAM load (~8us) during the preamble/idx load instead
            # of stalling the first real gather
            idxw = idxp.tile([P, 1], mybir.dt.int16)
            nc.gpsimd.memset(idxw[:], 0)
            warm = idxp.tile([P, 1, FEAT], mybir.dt.bfloat16)
            nc.gpsimd.dma_gather(warm[:], t_feat[:], idxw[:], 16, 16, FEAT,
                                 queue_num=0)
            nc.sync.dma_start(out=idx[:], in_=t_idx[:])
            acc = accp.tile([P, TILES, FEAT], mybir.dt.bfloat16)

            pend_lo = TILES  # writeback merge: pending final range [lo, hi)
            pend_hi = TILES
            for k, (cs, ce) in enumerate(chunks):
                w = ce - cs
                if ce <= TILES:                          # round 0: direct
                    gout = acc[:, cs:ce, :]
                else:
                    g = gp.tile([P, G, FEAT], mybir.dt.bfloat16, tag="g")
                    gout = g[:, :w, :]
                nc.gpsimd.dma_gather(gout, t_feat[:], idx[:, cs * 8:ce * 8],
                                     w * P, w * P, FEAT,
                                     queue_num=(k + 1) % 4)
                # max pieces for rounds j>=1 covered by the chunk; round 1
                # (which touches every deg>=2 tile exactly once) also folds
                # in the relu, later rounds use the faster plain max
                for j in range(1, maxd):
                    a = max(cs, int(bases[j]))
                    b = min(ce, int(bases[j]) + T[j])
                    if a < b:
                        t0 = a - int(bases[j])
                        L = b - a
                        if j == 1:
                            nc.vector.scalar_tensor_tensor(
                                out=acc[:, t0:t0 + L, :],
                                in0=acc[:, t0:t0 + L, :], scalar=0.0,
                                in1=g[:, a - cs:b - cs, :], op0=mx, op1=mx)
                        else:
                            nc.vector.tensor_tensor(
                                out=acc[:, t0:t0 + L, :],
                                in0=acc[:, t0:t0 + L, :],
                                in1=g[:, a - cs:b - cs, :], op=mx)
                # writebacks for rounds that completed with this chunk
                for j in range(maxd):
                    if end_chunk.get(j) != k:
                        continue
                    lo = T[j + 1] if j + 1 < maxd else 0
                    if j == 0 and lo < TILES:
                        # round-0-only tiles: relu never fused -> Act engine
                        nc.scalar.activation(acc[:, lo:TILES, :],
                                             acc[:, lo:TILES, :], relu)
                    pend_lo = min(pend_lo, lo)
                    final = k == len(chunks) - 1
                    if pend_hi - pend_lo >= WMIN or (final and
                                                     pend_hi > pend_lo):
                        nc.sync.dma_start(out=t_oe[:, pend_lo:pend_hi, :],
                                          in_=acc[:, pend_lo:pend_hi, :])
                        pend_hi = pend_lo
                if k == len(chunks) - 1 and pend_hi > pend_lo:
                    nc.sync.dma_start(out=t_oe[:, pend_lo:pend_hi, :],
                                      in_=acc[:, pend_lo:pend_hi, :])
    nc.compile()
    return nc


def _unshard(results, orders, feat_centers):
    out = np.empty((N_NODES, FEAT), np.float32)
    out[:N_CENTERS] = feat_centers                       # centers: exact copy
    for c in range(NCORES):
        oe = np.asarray(results[c]["out_edge"])          # [P, TILES, FEAT]
        vals = oe.transpose(1, 0, 2).reshape(NPOS, FEAT)  # position-major
        rows = N_CENTERS + c * RC + orders[c]            # position q -> row
        out[rows] = vals[:RC].astype(np.float32)
    return out


def kernel(feat, center_idx, edge_src, edge_dst, n_nodes, _trace=False):
    assert int(n_nodes) == N_NODES
    feat = np.ascontiguousarray(np.asarray(feat, np.float32))
    center_idx = np.asarray(center_idx, np.int64)

    # centers: out[center_idx] = feat, handled fully on the host (pure copy)
    feat_centers = np.zeros((N_CENTERS, FEAT), np.float32)
    feat_centers[center_idx] = feat

    T, bases, C, tables, idx_planes, orders = _build_plan(edge_src, edge_dst,
                                                          feat)
    nc = _build_bass(T, bases, C)

    if _trace:
        _install_profile_hook()
    import concourse.bass_utils as bass_utils
    bass_utils.upload_artifacts = lambda tmpdir: f"file://{tmpdir}"
    from concourse.bass_utils import run_bass_kernel_spmd

    in_maps = [{"feat_tbl": tables[c], "idxs": idx_planes[c]}
               for c in range(NCORES)]
    kw = dict(trace=True) if _trace else {}
    res = run_bass_kernel_spmd(nc, in_maps, list(range(NCORES)), **kw)

    out = _unshard(res.results, orders, feat_centers)
    if _trace:
        return out, res
    return out


# revision 28
# speedup vs baseline: 1.0570x; 1.0035x over previous
"""ColorUnpool (gather + segment-max + relu) as an 8-core Trainium2 Bass kernel.

Reference semantics:
    out = zeros([200000, 256]);  out[center_idx] = feat            # centers
    seg = segment_max(feat[edge_src], edge_dst)                    # edges
    out[r] = max(seg[r], 0) for rows r with >= 1 incoming edge

edge_dst only hits rows [50000, 200000) and center_idx only [0, 50000), so
the two regions are disjoint.  The center region is a pure host-side copy of
the input (no compute); the device computes the edge region only.

Device strategy (per core, dst rows split 8 ways -> 18750 rows/core):
  * The feat table is compacted per core to its ~31.6k distinct src rows
    (< 32768) so gather indices fit in int16, and the gather runs as
    1024-index `dma_gather` instructions (the HW cap) round-robined over
    all 4 SWDGE queues -- descriptor generation for different queues runs
    concurrently on the Q7 cores, which quarters the ~7.7ns/row software
    DGE cost that serialized the old per-column indirect-DMA design.
  * Pair packing: a greedy matching gives ~90% of deg>=2 rows one pair of
    same-row srcs placed in adjacent table rows (2q, 2q+1), so one 1KB
    descriptor (48ns DMA vs 2x33.6ns, one Q7 idx instead of two) fetches
    two edges at once.  Rows are split into block A (paired rows, degree
    desc) and block BC (unpaired deg>=2, then deg<=1 rows, degree desc);
    each block keeps a tight monotone per-tile round structure.
  * Column layout: one pair round over all A tiles (slot q fetches pair q;
    zero pads) whose fused DVE op  acc = max(max(gA, 0), gB)  initializes
    the accumulator relu included; a direct round 0 for BC tiles gathers
    edge 0 straight into the accumulator; then A single rounds and BC
    single rounds (prefix per-tile-max, ZID pads) fold in with DVE maxes
    (BC round 1 fuses the relu, BC tiles with deg<=1 get an
    Activation-engine relu).  Finished tiles are written back as soon as
    their last round completes, overlapping output DMA with gathers.
  * A dummy 16-idx gather triggers the Q7 mlp library IRAM load during
    the preamble; feat is bf16 on device (rel err ~4e-3 << 2e-2 gate);
    the host un-permutes rows and upcasts to f32.
"""

import sys
import types

import numpy as np
import ml_dtypes

sys.path.insert(0, "/opt/trn_rl_repo")

N_NODES = 200000
N_CENTERS = 50000
FEAT = 256
NCORES = 8
P = 128

R_EDGE = N_NODES - N_CENTERS          # 150000 edge-target rows
RC = R_EDGE // NCORES                 # 18750 edge rows per core
TILES = (RC + P - 1) // P             # 147 tiles of 128 rows
NPOS = TILES * P                      # 18816 padded row slots
TBL = 32768                           # per-core compact feat table rows
ZID = TBL - 1                         # zero single row (table zero-padded)
NPAIR = TBL // 2                      # pair view [16384, 512]
G = 8                                 # gather chunk width (cols); HW caps a
                                      # single dma_gather at 1024 indices
WMIN = 8                              # min writeback width (tiles)


def _install_profile_hook():
    """Provide antenv.axon_hooks (missing on this image) so that
    run_bass_kernel_spmd(trace=True) can profile via the axon .so."""
    try:
        import antenv
        if "antenv.axon_hooks" in sys.modules:
            return
        from trn_agent_boot.trn_boot import _ntff_profile_via_ctypes
        mod = types.ModuleType("antenv.axon_hooks")
        hook = _ntff_profile_via_ctypes("/opt/axon/libaxon_pjrt.so")
        mod.get_axon_ntff_profile_hook = lambda: hook
        mod.set_axon_ntff_profile_hook = lambda h: None
        sys.modules["antenv.axon_hooks"] = mod
        antenv.axon_hooks = mod
    except Exception:
        pass


def _prep_core(ld, ss):
    """CSR + greedy one-pair-per-row matching for one core."""
    deg = np.bincount(ld, minlength=RC)
    eo = np.argsort(ld, kind="stable")
    ss_sorted = ss[eo]
    starts = np.concatenate([[0], np.cumsum(deg)[:-1]])
    uniq, inv = np.unique(ss_sorted, return_inverse=True)
    U = len(uniq)
    assert U + 2 < TBL, f"{U} distinct srcs > int16 budget"

    bydeg = np.argsort(-deg, kind="stable")
    free = np.ones(U, bool)
    row_pair = [None] * RC                # (src_a, src_b) or None
    row_srcs = [None] * RC                # distinct src ids
    for r in bydeg:
        d = int(deg[r])
        if d == 0:
            row_srcs[r] = np.empty(0, np.int64)
            continue
        srcs = np.unique(inv[starts[r]:starts[r] + d])
        row_srcs[r] = srcs
        if d >= 2:
            cand = [int(s) for s in srcs if free[s]]
            if len(cand) >= 2:
                a, b = cand[0], cand[1]
                free[a] = False
                free[b] = False
                row_pair[r] = (a, b)
    return dict(deg=deg, uniq=uniq, row_pair=row_pair, row_srcs=row_srcs,
                bydeg=bydeg)


def _build_plan(edge_src, edge_dst, feat):
    """Host preprocessing.  Returns (segs, col_base, C, nA, TA, TBC,
    tables, pair_tables, idx_planes, orders)."""
    edge_src = np.asarray(edge_src, np.int64)
    edge_dst = np.asarray(edge_dst, np.int64)
    local_dst = edge_dst - N_CENTERS
    assert local_dst.min() >= 0 and local_dst.max() < R_EDGE
    core_of = local_dst // RC

    cores = []
    for c in range(NCORES):
        m = core_of == c
        cores.append(_prep_core((local_dst[m] % RC).astype(np.int64),
                                edge_src[m].astype(np.int64)))

    # block A: paired rows (degree desc), capped at a shared whole-tile
    # count; demoted rows lose their pair and join BC
    nA = min(sum(1 for r in range(RC) if pc["row_pair"][r] is not None)
             for pc in cores) // P
    M = nA * P
    assert nA >= 1

    percore = []
    for pc in cores:
        a_rows = [r for r in pc["bydeg"] if pc["row_pair"][r] is not None]
        for r in a_rows[M:]:
            pc["row_pair"][r] = None                     # demote
        a_rows = a_rows[:M]
        bc_rows = [r for r in pc["bydeg"] if pc["row_pair"][r] is None]
        order = np.array(a_rows + bc_rows, np.int64)

        # table: pair q of row a_rows[q] -> rows (2q, 2q+1); rest appended
        tbl_row = np.full(len(pc["uniq"]), -1, np.int64)
        for q, r in enumerate(a_rows):
            a, b = pc["row_pair"][r]
            tbl_row[a] = 2 * q
            tbl_row[b] = 2 * q + 1
        rest = np.nonzero(tbl_row < 0)[0]
        assert 2 * M + len(rest) <= TBL - 1
        tbl_row[rest] = 2 * M + np.arange(len(rest))

        # per-position singles (ragged): A rows exclude their pair
        sing = []
        for i, r in enumerate(order):
            srcs = pc["row_srcs"][r]
            if i < M:
                a, b = pc["row_pair"][r]
                srcs = srcs[(srcs != a) & (srcs != b)]
            sing.append(tbl_row[srcs])
        s_len = np.array([len(x) for x in sing] + [0] * (NPOS - RC))
        s_flat = np.concatenate(sing)
        if len(s_flat) == 0:
            s_flat = np.zeros(1, np.int64)
        s_starts = np.concatenate([[0], np.cumsum(s_len)[:-1]])
        SA = s_len[:M].reshape(nA, P).max(1)             # A tile max singles
        SBC = s_len[M:].reshape(TILES - nA, P).max(1)    # BC tile max
        percore.append(dict(order=order, tbl_row=tbl_row, uniq=pc["uniq"],
                            s_len=s_len, s_flat=s_flat, s_starts=s_starts,
                            SA=SA, SBC=SBC))

    def _pwidth(arrs, j):
        # pad-prefix width: 1 + last tile index with value > j (any core);
        # robust to small non-monotonicity (distinct-src count vs degree)
        w = 0
        for a in arrs:
            nz = np.nonzero(a > j)[0]
            if len(nz):
                w = max(w, int(nz[-1]) + 1)
        return w

    TA = []                                              # A single rounds
    for us in range(max(int(pc["SA"].max()) for pc in percore)):
        TA.append(_pwidth([pc["SA"] for pc in percore], us))
    TBC = [TILES - nA]                                   # BC round 0: all
    for js in range(1, max(int(pc["SBC"].max()) for pc in percore)):
        TBC.append(_pwidth([pc["SBC"] for pc in percore], js))

    segs = [("p", 0, nA), ("s0", 0, TILES - nA)]
    segs += [("sa", us, TA[us]) for us in range(len(TA)) if TA[us] > 0]
    segs += [("sb", js, TBC[js]) for js in range(1, len(TBC)) if TBC[js] > 0]
    col_base = np.concatenate([[0], np.cumsum([n for _, _, n in segs])])
    C = int(col_base[-1])

    tables, pair_tables, idx_planes, orders = [], [], [], []
    for pc in percore:
        s_len, s_flat, s_starts = pc["s_len"], pc["s_flat"], pc["s_starts"]
        vals = np.zeros(C * P, np.int64)
        for si, (kind, j, n) in enumerate(segs):
            base = int(col_base[si]) * P
            if kind == "p":
                vals[base:base + n * P] = np.arange(M)   # pair q at slot q
                continue
            if kind == "s0":
                qpos = np.arange(M, M + n * P)
                j = 0
            elif kind == "sa":
                qpos = np.arange(n * P)
            else:
                qpos = np.arange(M, M + n * P)
            has = s_len[qpos] > j
            v = np.where(has, s_flat[np.minimum(s_starts[qpos] + j,
                                                len(s_flat) - 1)], ZID)
            vals[base:base + n * P] = v
        plane16 = vals.astype(np.int16).reshape(C * 8, 16).T
        idx_planes.append(np.ascontiguousarray(np.tile(plane16, (8, 1))))

        tbl = np.zeros((TBL, FEAT), ml_dtypes.bfloat16)
        tbl[pc["tbl_row"]] = feat[pc["uniq"]].astype(ml_dtypes.bfloat16)
        tables.append(tbl)
        pair_tables.append(tbl.reshape(NPAIR, 2 * FEAT))
        orders.append(pc["order"])
    return (segs, col_base, C, nA, TA, TBC, tables, pair_tables,
            idx_planes, orders)


def _build_bass(segs, col_base, C, nA, TA, TBC):
    import concourse.bacc as bacc
    import concourse.mybir as mybir
    import concourse.tile as tile

    nc = bacc.Bacc("TRN2", target_bir_lowering=False, debug=False,
                   num_devices=NCORES, num_swdge_queues=4)
    t_feat = nc.dram_tensor("feat_tbl", [TBL, FEAT], mybir.dt.bfloat16,
                            kind="ExternalInput")
    t_featp = nc.dram_tensor("feat_tblp", [NPAIR, 2 * FEAT],
                             mybir.dt.bfloat16, kind="ExternalInput")
    t_idx = nc.dram_tensor("idxs", [P, C * 8], mybir.dt.int16,
                           kind="ExternalInput")
    t_oe = nc.dram_tensor("out_edge", [P, TILES, FEAT], mybir.dt.bfloat16,
                          kind="ExternalOutput")

    mx = mybir.AluOpType.max
    relu = mybir.ActivationFunctionType.Relu
    TA0 = TA[0] if TA else 0
    TBC1 = TBC[1] if len(TBC) > 1 else 0

    seg_rng = [(int(col_base[si]), int(col_base[si + 1]))
               for si in range(len(segs))]
    # chunk regions: pair | direct | singles (sa+sb contiguous, same elem)
    regions = []
    for si, (kind, j, n) in enumerate(segs):
        knd = "s" if kind in ("sa", "sb") else kind
        lo, hi = seg_rng[si]
        if regions and regions[-1][2] == knd and regions[-1][1] == lo:
            regions[-1] = (regions[-1][0], hi, knd)
        else:
            regions.append((lo, hi, knd))
    chunks = [(s, min(s + G, hi), kind)
              for lo, hi, kind in regions for s in range(lo, hi, G)]

    with tile.TileContext(nc) as tc:
        with tc.tile_pool(name="idxp", bufs=1) as idxp, \
             tc.tile_pool(name="accp", bufs=1) as accp, \
             tc.tile_pool(name="gp", bufs=8) as gp, \
             tc.tile_pool(name="pp", bufs=4) as pp:
            idx = idxp.tile([P, C * 8], mybir.dt.int16)
            # dummy 16-idx gather with no data deps: triggers the Q7 mlp
            # library IRAM load during the preamble/idx load
            idxw = idxp.tile([P, 1], mybir.dt.int16)
            nc.gpsimd.memset(idxw[:], 0)
            warm = idxp.tile([P, 1, FEAT], mybir.dt.bfloat16)
            nc.gpsimd.dma_gather(warm[:], t_feat[:], idxw[:], 16, 16, FEAT,
                                 queue_num=0)
            nc.sync.dma_start(out=idx[:], in_=t_idx[:])
            acc = accp.tile([P, TILES, FEAT], mybir.dt.bfloat16)

            pend = []          # pending finalized tile ranges [lo, hi)

            def add_final(lo, hi, force=False):
                if lo < hi:
                    if pend and pend[-1][1] == lo:
                        pend[-1] = (pend[-1][0], hi)
                    elif pend and pend[-1][0] == hi:
                        pend[-1] = (lo, pend[-1][1])
                    else:
                        pend.append((lo, hi))
                keep = []
                for lo, hi in pend:
                    if hi - lo >= WMIN or force:
                        nc.sync.dma_start(out=t_oe[:, lo:hi, :],
                                          in_=acc[:, lo:hi, :])
                    else:
                        keep.append((lo, hi))
                pend[:] = keep

            for k, (cs, ce, kind) in enumerate(chunks):
                w = ce - cs
                qn = (k + 1) % 4
                if kind == "p":
                    g = pp.tile([P, G, 2 * FEAT], mybir.dt.bfloat16,
                                tag="gpair")
                    nc.gpsimd.dma_gather(g[:, :w, :], t_featp[:],
                                         idx[:, cs * 8:ce * 8],
                                         w * P, w * P, 2 * FEAT,
                                         queue_num=qn)
                    # acc[t] = max(max(gA, 0), gB): init + relu in one op
                    nc.vector.scalar_tensor_tensor(
                        out=acc[:, cs:ce, :],
                        in0=g[:, :w, 0:FEAT], scalar=0.0,
                        in1=g[:, :w, FEAT:2 * FEAT], op0=mx, op1=mx)
                    # A tiles with no singles are done now
                    add_final(max(cs, TA0), ce)
                    continue
                if kind == "s0":
                    lo_t = nA + (cs - seg_rng[1][0])
                    hi_t = nA + (ce - seg_rng[1][0])
                    nc.gpsimd.dma_gather(acc[:, lo_t:hi_t, :], t_feat[:],
                                         idx[:, cs * 8:ce * 8],
                                         w * P, w * P, FEAT, queue_num=qn)
                    # BC tiles with deg<=1: relu on Act, then final
                    lo = max(lo_t, nA + TBC1)
                    if lo < hi_t:
                        nc.scalar.activation(acc[:, lo:hi_t, :],
                                             acc[:, lo:hi_t, :], relu)
                        add_final(lo, hi_t)
                    continue
                g = gp.tile([P, G, FEAT], mybir.dt.bfloat16, tag="g")
                nc.gpsimd.dma_gather(g[:, :w, :], t_feat[:],
                                     idx[:, cs * 8:ce * 8],
                                     w * P, w * P, FEAT, queue_num=qn)
                for si, (knd, j, n) in enumerate(segs):
                    if knd not in ("sa", "sb"):
                        continue
                    a = max(cs, seg_rng[si][0])
                    b = min(ce, seg_rng[si][1])
                    if a >= b:
                        continue
                    go = a - cs
                    L = b - a
                    if knd == "sa":
                        tp = a - seg_rng[si][0]
                        nxt = TA[j + 1] if j + 1 < len(TA) else 0
                        nc.vector.tensor_tensor(
                            out=acc[:, tp:tp + L, :],
                            in0=acc[:, tp:tp + L, :],
                            in1=g[:, go:go + L, :], op=mx)
                        add_final(max(tp, nxt), tp + L)
                    else:
                        tp = nA + (a - seg_rng[si][0])
                        nxt = TBC[j + 1] if j + 1 < len(TBC) else 0
                        if j == 1:
                            # first BC reduction: fold the relu in
                            nc.vector.scalar_tensor_tensor(
                                out=acc[:, tp:tp + L, :],
                                in0=acc[:, tp:tp + L, :], scalar=0.0,
                                in1=g[:, go:go + L, :], op0=mx, op1=mx)
                        else:
                            nc.vector.tensor_tensor(
                                out=acc[:, tp:tp + L, :],
                                in0=acc[:, tp:tp + L, :],
                                in1=g[:, go:go + L, :], op=mx)
                        add_final(max(tp, nA + nxt), tp + L)
            add_final(0, 0, force=True)
    nc.compile()
    return nc


def _unshard(results, orders, feat_centers):
    out = np.empty((N_NODES, FEAT), np.float32)
    out[:N_CENTERS] = feat_centers                       # centers: exact copy
    for c in range(NCORES):
        oe = np.asarray(results[c]["out_edge"])          # [P, TILES, FEAT]
        vals = oe.transpose(1, 0, 2).reshape(NPOS, FEAT)  # position-major
        rows = N_CENTERS + c * RC + orders[c]            # position q -> row
        out[rows] = vals[:RC].astype(np.float32)
    return out


def kernel(feat, center_idx, edge_src, edge_dst, n_nodes, _trace=False):
    assert int(n_nodes) == N_NODES
    feat = np.ascontiguousarray(np.asarray(feat, np.float32))
    center_idx = np.asarray(center_idx, np.int64)

    # centers: out[center_idx] = feat, handled fully on the host (pure copy)
    feat_centers = np.zeros((N_CENTERS, FEAT), np.float32)
    feat_centers[center_idx] = feat

    (segs, col_base, C, nA, TA, TBC, tables, pair_tables, idx_planes,
     orders) = _build_plan(edge_src, edge_dst, feat)
    nc = _build_bass(segs, col_base, C, nA, TA, TBC)

    if _trace:
        _install_profile_hook()
    import concourse.bass_utils as bass_utils
    bass_utils.upload_artifacts = lambda tmpdir: f"file://{tmpdir}"
    from concourse.bass_utils import run_bass_kernel_spmd

    in_maps = [{"feat_tbl": tables[c], "feat_tblp": pair_tables[c],
                "idxs": idx_planes[c]} for c in range(NCORES)]
    kw = dict(trace=True) if _trace else {}
    res = run_bass_kernel_spmd(nc, in_maps, list(range(NCORES)), **kw)

    out = _unshard(res.results, orders, feat_centers)
    if _trace:
        return out, res
    return out


# revision 29
# speedup vs baseline: 1.1418x; 1.0802x over previous
"""ColorUnpool (gather + segment-max + relu) as an 8-core Trainium2 Bass kernel.

Reference semantics:
    out = zeros([200000, 256]);  out[center_idx] = feat            # centers
    seg = segment_max(feat[edge_src], edge_dst)                    # edges
    out[r] = max(seg[r], 0) for rows r with >= 1 incoming edge

edge_dst only hits rows [50000, 200000) and center_idx only [0, 50000), so
the two regions are disjoint.  The center region is a pure host-side copy of
the input (no compute); the device computes the edge region only.

Device strategy (per core, dst rows split 8 ways -> 18750 rows/core):
  * The feat table is compacted per core to its ~31.6k distinct src rows
    (< 32768) so gather indices fit in int16, and the gather runs as
    1024-index `dma_gather` instructions (the HW cap) round-robined over
    all 4 SWDGE queues -- descriptor generation for different queues runs
    concurrently on the Q7 cores, which quarters the ~7.7ns/row software
    DGE cost that serialized the old per-column indirect-DMA design.
  * Pair packing: a greedy matching gives ~90% of deg>=2 rows one pair of
    same-row srcs placed in adjacent table rows (2q, 2q+1), so one 1KB
    descriptor (48ns DMA vs 2x33.6ns, one Q7 idx instead of two) fetches
    two edges at once.  Rows are split into block A (paired rows, degree
    desc) and block BC (unpaired deg>=2, then deg<=1 rows, degree desc);
    each block keeps a tight monotone per-tile round structure.
  * Column layout: one pair round over all A tiles (slot q fetches pair q;
    zero pads) whose fused DVE op  acc = max(max(gA, 0), gB)  initializes
    the accumulator relu included; a direct round 0 for BC tiles gathers
    edge 0 straight into the accumulator; then A single rounds and BC
    single rounds (prefix per-tile-max, ZID pads) fold in with DVE maxes
    (BC round 1 fuses the relu, BC tiles with deg<=1 get an
    Activation-engine relu).  Finished tiles are written back as soon as
    their last round completes, overlapping output DMA with gathers.
  * A dummy 16-idx gather triggers the Q7 mlp library IRAM load during
    the preamble; feat is bf16 on device (rel err ~4e-3 << 2e-2 gate);
    the host un-permutes rows and upcasts to f32.
"""

import sys
import types

import numpy as np
import ml_dtypes

sys.path.insert(0, "/opt/trn_rl_repo")

N_NODES = 200000
N_CENTERS = 50000
FEAT = 256
NCORES = 8
P = 128

R_EDGE = N_NODES - N_CENTERS          # 150000 edge-target rows
RC = R_EDGE // NCORES                 # 18750 edge rows per core
TILES = (RC + P - 1) // P             # 147 tiles of 128 rows
NPOS = TILES * P                      # 18816 padded row slots
TBL = 32768                           # per-core compact feat table rows
ZID = TBL - 1                         # zero single row (table zero-padded)
NPAIR = TBL // 2                      # pair view [16384, 512]
G = 8                                 # gather chunk width (cols); HW caps a
                                      # single dma_gather at 1024 indices
WMIN = 8                              # min writeback width (tiles)


def _install_profile_hook():
    """Provide antenv.axon_hooks (missing on this image) so that
    run_bass_kernel_spmd(trace=True) can profile via the axon .so."""
    try:
        import antenv
        if "antenv.axon_hooks" in sys.modules:
            return
        from trn_agent_boot.trn_boot import _ntff_profile_via_ctypes
        mod = types.ModuleType("antenv.axon_hooks")
        hook = _ntff_profile_via_ctypes("/opt/axon/libaxon_pjrt.so")
        mod.get_axon_ntff_profile_hook = lambda: hook
        mod.set_axon_ntff_profile_hook = lambda h: None
        sys.modules["antenv.axon_hooks"] = mod
        antenv.axon_hooks = mod
    except Exception:
        pass


def _prep_core(ld, ss):
    """CSR + greedy one-pair-per-row matching for one core."""
    deg = np.bincount(ld, minlength=RC)
    eo = np.argsort(ld, kind="stable")
    ss_sorted = ss[eo]
    starts = np.concatenate([[0], np.cumsum(deg)[:-1]])
    uniq, inv = np.unique(ss_sorted, return_inverse=True)
    U = len(uniq)
    assert U + 2 < TBL, f"{U} distinct srcs > int16 budget"

    bydeg = np.argsort(-deg, kind="stable")
    free = np.ones(U, bool)
    row_pair = [None] * RC                # (src_a, src_b) or None
    row_srcs = [None] * RC                # distinct src ids
    for r in bydeg:
        d = int(deg[r])
        if d == 0:
            row_srcs[r] = np.empty(0, np.int64)
            continue
        srcs = np.unique(inv[starts[r]:starts[r] + d])
        row_srcs[r] = srcs
        if d >= 2:
            cand = [int(s) for s in srcs if free[s]]
            if len(cand) >= 2:
                a, b = cand[0], cand[1]
                free[a] = False
                free[b] = False
                row_pair[r] = (a, b)
    return dict(deg=deg, uniq=uniq, row_pair=row_pair, row_srcs=row_srcs,
                bydeg=bydeg)


def _build_plan(edge_src, edge_dst, feat):
    """Host preprocessing.  Returns (segs, col_base, C, nA, TA, TBC,
    tables, pair_tables, idx_planes, orders)."""
    edge_src = np.asarray(edge_src, np.int64)
    edge_dst = np.asarray(edge_dst, np.int64)
    local_dst = edge_dst - N_CENTERS
    assert local_dst.min() >= 0 and local_dst.max() < R_EDGE
    core_of = local_dst // RC

    cores = []
    for c in range(NCORES):
        m = core_of == c
        cores.append(_prep_core((local_dst[m] % RC).astype(np.int64),
                                edge_src[m].astype(np.int64)))

    # block A: paired rows (degree desc), capped at a shared whole-tile
    # count; demoted rows lose their pair and join BC
    nA = min(sum(1 for r in range(RC) if pc["row_pair"][r] is not None)
             for pc in cores) // P
    M = nA * P
    assert nA >= 1

    percore = []
    for pc in cores:
        a_rows = [r for r in pc["bydeg"] if pc["row_pair"][r] is not None]
        for r in a_rows[M:]:
            pc["row_pair"][r] = None                     # demote
        a_rows = a_rows[:M]
        bc_rows = [r for r in pc["bydeg"] if pc["row_pair"][r] is None]
        order = np.array(a_rows + bc_rows, np.int64)

        # table: pair q of row a_rows[q] -> rows (2q, 2q+1); rest appended
        tbl_row = np.full(len(pc["uniq"]), -1, np.int64)
        for q, r in enumerate(a_rows):
            a, b = pc["row_pair"][r]
            tbl_row[a] = 2 * q
            tbl_row[b] = 2 * q + 1
        rest = np.nonzero(tbl_row < 0)[0]
        assert 2 * M + len(rest) <= TBL - 1
        tbl_row[rest] = 2 * M + np.arange(len(rest))

        # per-position singles (ragged): A rows exclude their pair
        sing = []
        for i, r in enumerate(order):
            srcs = pc["row_srcs"][r]
            if i < M:
                a, b = pc["row_pair"][r]
                srcs = srcs[(srcs != a) & (srcs != b)]
            sing.append(tbl_row[srcs])
        s_len = np.array([len(x) for x in sing] + [0] * (NPOS - RC))
        s_flat = np.concatenate(sing)
        if len(s_flat) == 0:
            s_flat = np.zeros(1, np.int64)
        s_starts = np.concatenate([[0], np.cumsum(s_len)[:-1]])
        SA = s_len[:M].reshape(nA, P).max(1)             # A tile max singles
        SBC = s_len[M:].reshape(TILES - nA, P).max(1)    # BC tile max
        percore.append(dict(order=order, tbl_row=tbl_row, uniq=pc["uniq"],
                            s_len=s_len, s_flat=s_flat, s_starts=s_starts,
                            SA=SA, SBC=SBC))

    def _pwidth(arrs, j):
        # pad-prefix width: 1 + last tile index with value > j (any core);
        # robust to small non-monotonicity (distinct-src count vs degree)
        w = 0
        for a in arrs:
            nz = np.nonzero(a > j)[0]
            if len(nz):
                w = max(w, int(nz[-1]) + 1)
        return w

    TA = []                                              # A single rounds
    for us in range(max(int(pc["SA"].max()) for pc in percore)):
        TA.append(_pwidth([pc["SA"] for pc in percore], us))
    TBC = [TILES - nA]                                   # BC round 0: all
    for js in range(1, max(int(pc["SBC"].max()) for pc in percore)):
        TBC.append(_pwidth([pc["SBC"] for pc in percore], js))

    # sb before sa: the wide BC round-1 writeback lands mid-stream and the
    # narrow width-1 A tail rounds close out the pipeline
    segs = [("p", 0, nA), ("s0", 0, TILES - nA)]
    segs += [("sb", js, TBC[js]) for js in range(1, len(TBC)) if TBC[js] > 0]
    segs += [("sa", us, TA[us]) for us in range(len(TA)) if TA[us] > 0]
    col_base = np.concatenate([[0], np.cumsum([n for _, _, n in segs])])
    C = int(col_base[-1])

    tables, pair_tables, idx_planes, orders = [], [], [], []
    for pc in percore:
        s_len, s_flat, s_starts = pc["s_len"], pc["s_flat"], pc["s_starts"]
        vals = np.zeros(C * P, np.int64)
        for si, (kind, j, n) in enumerate(segs):
            base = int(col_base[si]) * P
            if kind == "p":
                vals[base:base + n * P] = np.arange(M)   # pair q at slot q
                continue
            if kind == "s0":
                qpos = np.arange(M, M + n * P)
                j = 0
            elif kind == "sa":
                qpos = np.arange(n * P)
            else:
                qpos = np.arange(M, M + n * P)
            has = s_len[qpos] > j
            v = np.where(has, s_flat[np.minimum(s_starts[qpos] + j,
                                                len(s_flat) - 1)], ZID)
            vals[base:base + n * P] = v
        plane16 = vals.astype(np.int16).reshape(C * 8, 16).T
        idx_planes.append(np.ascontiguousarray(np.tile(plane16, (8, 1))))

        tbl = np.zeros((TBL, FEAT), ml_dtypes.bfloat16)
        tbl[pc["tbl_row"]] = feat[pc["uniq"]].astype(ml_dtypes.bfloat16)
        tables.append(tbl)
        pair_tables.append(tbl.reshape(NPAIR, 2 * FEAT))
        orders.append(pc["order"])
    return (segs, col_base, C, nA, TA, TBC, tables, pair_tables,
            idx_planes, orders)


def _build_bass(segs, col_base, C, nA, TA, TBC):
    import concourse.bacc as bacc
    import concourse.mybir as mybir
    import concourse.tile as tile

    nc = bacc.Bacc("TRN2", target_bir_lowering=False, debug=False,
                   num_devices=NCORES, num_swdge_queues=4)
    t_feat = nc.dram_tensor("feat_tbl", [TBL, FEAT], mybir.dt.bfloat16,
                            kind="ExternalInput")
    t_featp = nc.dram_tensor("feat_tblp", [NPAIR, 2 * FEAT],
                             mybir.dt.bfloat16, kind="ExternalInput")
    t_idx = nc.dram_tensor("idxs", [P, C * 8], mybir.dt.int16,
                           kind="ExternalInput")
    t_oe = nc.dram_tensor("out_edge", [P, TILES, FEAT], mybir.dt.bfloat16,
                          kind="ExternalOutput")

    mx = mybir.AluOpType.max
    relu = mybir.ActivationFunctionType.Relu
    TA0 = TA[0] if TA else 0
    TBC1 = TBC[1] if len(TBC) > 1 else 0

    seg_rng = [(int(col_base[si]), int(col_base[si + 1]))
               for si in range(len(segs))]
    # chunk regions: pair | direct | singles (sa+sb contiguous, same elem)
    regions = []
    for si, (kind, j, n) in enumerate(segs):
        knd = "s" if kind in ("sa", "sb") else kind
        lo, hi = seg_rng[si]
        if regions and regions[-1][2] == knd and regions[-1][1] == lo:
            regions[-1] = (regions[-1][0], hi, knd)
        else:
            regions.append((lo, hi, knd))
    chunks = [(s, min(s + G, hi), kind)
              for lo, hi, kind in regions for s in range(lo, hi, G)]

    with tile.TileContext(nc) as tc:
        with tc.tile_pool(name="idxp", bufs=1) as idxp, \
             tc.tile_pool(name="accp", bufs=1) as accp, \
             tc.tile_pool(name="gp", bufs=8) as gp, \
             tc.tile_pool(name="pp", bufs=4) as pp:
            idx = idxp.tile([P, C * 8], mybir.dt.int16)
            # dummy 16-idx gather with no data deps: triggers the Q7 mlp
            # library IRAM load during the preamble/idx load
            idxw = idxp.tile([P, 1], mybir.dt.int16)
            nc.gpsimd.memset(idxw[:], 0)
            warm = idxp.tile([P, 1, FEAT], mybir.dt.bfloat16)
            nc.gpsimd.dma_gather(warm[:], t_feat[:], idxw[:], 16, 16, FEAT,
                                 queue_num=0)
            nc.sync.dma_start(out=idx[:], in_=t_idx[:])
            acc = accp.tile([P, TILES, FEAT], mybir.dt.bfloat16)

            pend = []          # pending finalized tile ranges [lo, hi)

            def add_final(lo, hi, force=False):
                if lo < hi:
                    if pend and pend[-1][1] == lo:
                        pend[-1] = (pend[-1][0], hi)
                    elif pend and pend[-1][0] == hi:
                        pend[-1] = (lo, pend[-1][1])
                    else:
                        pend.append((lo, hi))
                keep = []
                for lo, hi in pend:
                    if hi - lo >= WMIN or force:
                        nc.sync.dma_start(out=t_oe[:, lo:hi, :],
                                          in_=acc[:, lo:hi, :])
                    else:
                        keep.append((lo, hi))
                pend[:] = keep

            for k, (cs, ce, kind) in enumerate(chunks):
                w = ce - cs
                qn = (k + 1) % 4
                if kind == "p":
                    g = pp.tile([P, G, 2 * FEAT], mybir.dt.bfloat16,
                                tag="gpair")
                    nc.gpsimd.dma_gather(g[:, :w, :], t_featp[:],
                                         idx[:, cs * 8:ce * 8],
                                         w * P, w * P, 2 * FEAT,
                                         queue_num=qn)
                    # acc[t] = max(max(gA, 0), gB): init + relu in one op
                    nc.vector.scalar_tensor_tensor(
                        out=acc[:, cs:ce, :],
                        in0=g[:, :w, 0:FEAT], scalar=0.0,
                        in1=g[:, :w, FEAT:2 * FEAT], op0=mx, op1=mx)
                    # A tiles with no singles are done now
                    add_final(max(cs, TA0), ce)
                    continue
                if kind == "s0":
                    lo_t = nA + (cs - seg_rng[1][0])
                    hi_t = nA + (ce - seg_rng[1][0])
                    nc.gpsimd.dma_gather(acc[:, lo_t:hi_t, :], t_feat[:],
                                         idx[:, cs * 8:ce * 8],
                                         w * P, w * P, FEAT, queue_num=qn)
                    # BC tiles with deg<=1: relu on Act, then final
                    lo = max(lo_t, nA + TBC1)
                    if lo < hi_t:
                        nc.scalar.activation(acc[:, lo:hi_t, :],
                                             acc[:, lo:hi_t, :], relu)
                        add_final(lo, hi_t)
                    continue
                g = gp.tile([P, G, FEAT], mybir.dt.bfloat16, tag="g")
                nc.gpsimd.dma_gather(g[:, :w, :], t_feat[:],
                                     idx[:, cs * 8:ce * 8],
                                     w * P, w * P, FEAT, queue_num=qn)
                for si, (knd, j, n) in enumerate(segs):
                    if knd not in ("sa", "sb"):
                        continue
                    a = max(cs, seg_rng[si][0])
                    b = min(ce, seg_rng[si][1])
                    if a >= b:
                        continue
                    go = a - cs
                    L = b - a
                    if knd == "sa":
                        tp = a - seg_rng[si][0]
                        nxt = TA[j + 1] if j + 1 < len(TA) else 0
                        nc.vector.tensor_tensor(
                            out=acc[:, tp:tp + L, :],
                            in0=acc[:, tp:tp + L, :],
                            in1=g[:, go:go + L, :], op=mx)
                        add_final(max(tp, nxt), tp + L)
                    else:
                        tp = nA + (a - seg_rng[si][0])
                        nxt = TBC[j + 1] if j + 1 < len(TBC) else 0
                        if j == 1:
                            # first BC reduction: fold the relu in
                            nc.vector.scalar_tensor_tensor(
                                out=acc[:, tp:tp + L, :],
                                in0=acc[:, tp:tp + L, :], scalar=0.0,
                                in1=g[:, go:go + L, :], op0=mx, op1=mx)
                        else:
                            nc.vector.tensor_tensor(
                                out=acc[:, tp:tp + L, :],
                                in0=acc[:, tp:tp + L, :],
                                in1=g[:, go:go + L, :], op=mx)
                        add_final(max(tp, nA + nxt), tp + L)
            add_final(0, 0, force=True)
    nc.compile()
    return nc


def _unshard(results, orders, feat_centers):
    out = np.empty((N_NODES, FEAT), np.float32)
    out[:N_CENTERS] = feat_centers                       # centers: exact copy
    for c in range(NCORES):
        oe = np.asarray(results[c]["out_edge"])          # [P, TILES, FEAT]
        vals = oe.transpose(1, 0, 2).reshape(NPOS, FEAT)  # position-major
        rows = N_CENTERS + c * RC + orders[c]            # position q -> row
        out[rows] = vals[:RC].astype(np.float32)
    return out


def kernel(feat, center_idx, edge_src, edge_dst, n_nodes, _trace=False):
    assert int(n_nodes) == N_NODES
    feat = np.ascontiguousarray(np.asarray(feat, np.float32))
    center_idx = np.asarray(center_idx, np.int64)

    # centers: out[center_idx] = feat, handled fully on the host (pure copy)
    feat_centers = np.zeros((N_CENTERS, FEAT), np.float32)
    feat_centers[center_idx] = feat

    (segs, col_base, C, nA, TA, TBC, tables, pair_tables, idx_planes,
     orders) = _build_plan(edge_src, edge_dst, feat)
    nc = _build_bass(segs, col_base, C, nA, TA, TBC)

    if _trace:
        _install_profile_hook()
    import concourse.bass_utils as bass_utils
    bass_utils.upload_artifacts = lambda tmpdir: f"file://{tmpdir}"
    from concourse.bass_utils import run_bass_kernel_spmd

    in_maps = [{"feat_tbl": tables[c], "feat_tblp": pair_tables[c],
                "idxs": idx_planes[c]} for c in range(NCORES)]
    kw = dict(trace=True) if _trace else {}
    res = run_bass_kernel_spmd(nc, in_maps, list(range(NCORES)), **kw)

    out = _unshard(res.results, orders, feat_centers)
    if _trace:
        return out, res
    return out
